# revision 3
# baseline (speedup 1.0000x reference)
"""Trainium2 Bass kernel for nn_DfOpStrided — v7 (multi-engine split).

Math (reference):
    x[t] = spec[:, 0, t, :96, :]                     (complex, [T, 96])
    spec_f[t] = sum_k c[t, k] * x[t + k - 4]         (complex MAC, zero-pad t<0)
    out[t] = alpha[t] * spec_f[t] + (1 - alpha[t]) * x[t]

Host-side fusion: out[t] = sum_k chat[t, k] * x[t+k-4] with
    chat[t, k] = alpha[t] * c[t, k]  (+ (1-alpha[t]) on Re(chat[t, 4]))
because tap k=4 multiplies x[t] itself.

Engine split (v3 was DVE-only and DVE-bound at ~63us busy):
  - DVE: tap 0-3 products (plain complex mult, 3 TT ops/tap, all 2x-aligned)
         + the final re/im combine.
  - GpSimd: tap-4 products + building the odd-parity shifted x copy on
    device (cuts xin DMA traffic; gp is alignment-agnostic anyway).
  - PE: sums the 5 tap-product planes via identity-stationary matmuls
    accumulating in PSUM (replaces the DVE accumulate-add chain).
  - ACT: PSUM (fp32) -> SBUF (fp16) copy, plus the x/ident DMA ring.
  - Sync: coef/y DMA ring (HWDGE).

Products per tap (plain complex, not Gauss — same DVE cols once the
Gauss coef-prep subtracts are counted, but one less x plane of DMA):
    P0 = cr*xr, P1 = ci*xi, P2 = cr*xi, P3 = ci*xr
    re = P0 - P1, im = P2 + P3   (combined on device from the PSUM sums)

Packing: row = one (batch, time-segment, freq-bin) triple, Wt=500 samples
per segment + 4-sample halo. 2*4*96 = 768 rows per core = six 128-row
tiles. Pure data-parallel over batch: 16 batches -> 8 cores x 2.
"""

import sys

sys.path.insert(0, "/opt/trn_rl_repo")

import numpy as np
from concourse import bass, bacc, tile, mybir
from concourse.bass_utils import run_bass_kernel_spmd

B, T, F, NDF, ORDER = 16, 2000, 481, 96, 5
NCORES = 8
BPC = B // NCORES  # batches per core
PAD = ORDER - 1  # causal zero-pad
Wt = 500  # samples per row segment
SEG = T // Wt  # segments per batch
XW = Wt + PAD  # x row window (halo)
P = 128
ROWS = BPC * SEG * NDF  # 768 rows per core
NT = ROWS // P  # 6 tiles per core

_cache: dict = {}


def _build():
    if "nc" in _cache:
        return _cache["nc"]
    f32 = mybir.dt.float32
    dt = mybir.dt.float16
    mult = mybir.AluOpType.mult
    nc = bacc.Bacc("TRN2", target_bir_lowering=False, debug=False, num_devices=NCORES)
    # x rows, partition-major: [P, NT, plane(xr,xi), XW]
    xin = nc.dram_tensor("xin", [P, NT, 2, XW], dt, kind="ExternalInput")
    # coef rows: [P, NT, tap, plane(cr,ci), Wt]
    coef = nc.dram_tensor("coef", [P, NT, ORDER, 2, Wt], dt, kind="ExternalInput")
    ident = nc.dram_tensor("ident", [P, P], dt, kind="ExternalInput")
    # y rows: [P, NT, plane(re,im), Wt]
    y = nc.dram_tensor("y", [P, NT, 2, Wt], dt, kind="ExternalOutput")

    with tile.TileContext(nc) as tc:
        with (
            tc.tile_pool(name="xp", bufs=1) as xpool,
            tc.tile_pool(name="cp", bufs=3) as cpool,
            tc.tile_pool(name="pp", bufs=2) as ppool,
            tc.psum_pool(name="ps", bufs=2) as pspool,
            tc.tile_pool(name="ab", bufs=2) as apool,
            tc.tile_pool(name="yp", bufs=2) as ypool,
        ):
            idt = xpool.tile([P, P], dt, tag="ident")
            nc.scalar.dma_start(out=idt[:, :], in_=ident[:, :])
            xt = xpool.tile([P, NT, 2, XW], dt, tag="x")
            nc.scalar.dma_start(out=xt[:, 0:3], in_=xin[:, 0:3])
            nc.scalar.dma_start(out=xt[:, 3:6], in_=xin[:, 3:6])
            # odd-parity shifted copy (for 4B-aligned 2x DVE reads at odd taps)
            xo = xpool.tile([P, NT, 2, XW], dt, tag="xodd")
            for i in range(NT):
                nc.gpsimd.tensor_copy(
                    out=xo[:, i, :, 0 : XW - 1], in_=xt[:, i, :, 1:XW]
                )
            for i in range(NT):
                ck = cpool.tile([P, ORDER, 2, Wt], dt, tag="ck")
                nc.sync.dma_start(out=ck[:, :], in_=coef[:, i])
                prods = []
                for k in range(ORDER):
                    pk = ppool.tile([P, 4, Wt], dt, tag=f"prod{k}")
                    eng = nc.vector if k < 4 else nc.gpsimd
                    par = k % 2
                    src = xt if par == 0 else xo
                    off = k - par  # even offset into src
                    x2 = src[:, i, :, off : off + Wt]  # [P, 2, Wt] (xr, xi)
                    xr = src[:, i, 0, off : off + Wt]
                    xi_ = src[:, i, 1, off : off + Wt]
                    # P0 = cr*xr, P1 = ci*xi in one 2-plane op
                    eng.tensor_tensor(pk[:, 0:2, :], ck[:, k, :, :], x2, mult)
                    eng.tensor_tensor(pk[:, 2, :], ck[:, k, 0, :], xi_, mult)
                    eng.tensor_tensor(pk[:, 3, :], ck[:, k, 1, :], xr, mult)
                    prods.append(pk)
                # one PSUM bank (512 fp32) per product plane: matmul outputs
                # must not cross bank boundaries
                ps = pspool.tile([P, 4, 512], f32, tag="psum")
                for k in range(ORDER):
                    for s in range(4):
                        nc.tensor.matmul(
                            ps[:, s, 0:Wt],
                            idt[:, :],
                            prods[k][:, s, :],
                            start=(k == 0),
                            stop=(k == ORDER - 1),
                        )
                abc = apool.tile([P, 4, Wt], dt, tag="abc")
                nc.scalar.activation(
                    abc[:, :], ps[:, :, 0:Wt], mybir.ActivationFunctionType.Copy
                )
                yt = ypool.tile([P, 2, Wt], dt, tag="y")
                nc.vector.tensor_tensor(
                    yt[:, 0, :], abc[:, 0, :], abc[:, 1, :], mybir.AluOpType.subtract
                )
                nc.vector.tensor_tensor(
                    yt[:, 1, :], abc[:, 2, :], abc[:, 3, :], mybir.AluOpType.add
                )
                nc.sync.dma_start(out=y[:, i], in_=yt[:, :])
    nc.compile()
    _cache["nc"] = nc
    return nc


def _host_prep(spec, coefs, alpha):
    """Build per-core xin/coef arrays (all cores at once)."""
    spec32 = np.asarray(spec, dtype=np.float32)
    coefs32 = np.asarray(coefs, dtype=np.float32)
    alpha32 = np.asarray(alpha, dtype=np.float32)

    x = spec32[:, 0, :, :NDF, :]  # [B, T, 96, 2]
    xpad = np.zeros((B, 2, NDF, PAD + T), dtype=np.float32)
    xpad[:, 0, :, PAD:] = x[..., 0].transpose(0, 2, 1)
    xpad[:, 1, :, PAD:] = x[..., 1].transpose(0, 2, 1)
    xpad = xpad.astype(np.float16)
    # per-segment windows with halo: [B, SEG, 2, 96, XW] -> rows (b, seg, f)
    xw = np.stack([xpad[:, :, :, s * Wt : s * Wt + XW] for s in range(SEG)], axis=1)
    xw = xw.transpose(0, 1, 3, 2, 4)  # [B, SEG, 96, 2, XW]
    xin_all = (
        xw.reshape(NCORES, NT, P, 2, XW).transpose(0, 2, 1, 3, 4).copy()
    )  # [NC, P, NT, 2, XW]

    a = alpha32[:, :, 0]  # [B, T]
    ca = coefs32 * a[:, :, None, None, None]  # [B, T, 5, 96, 2]
    ca[:, :, ORDER - 1, :, 0] += (1.0 - a)[:, :, None]
    ca = ca.astype(np.float16)
    cs = ca.reshape(B, SEG, Wt, ORDER, NDF, 2).transpose(
        0, 1, 4, 3, 5, 2
    )  # [B, SEG, 96, 5, 2, Wt]
    coef_all = (
        cs.reshape(NCORES, NT, P, ORDER, 2, Wt).transpose(0, 2, 1, 3, 4, 5).copy()
    )  # [NC, P, NT, 5, 2, Wt]

    ident = np.eye(P, dtype=np.float16)
    return xin_all, coef_all, ident


def kernel(spec, coefs, alpha, _bass_results_hook=None):
    nc = _build()
    xin_all, coef_all, ident = _host_prep(spec, coefs, alpha)

    core_ids = list(range(NCORES))
    in_maps = [
        {"xin": xin_all[c], "coef": coef_all[c], "ident": ident} for c in core_ids
    ]
    res = run_bass_kernel_spmd(nc, in_maps, core_ids)
    if _bass_results_hook is not None:
        _bass_results_hook(res)

    yy = np.stack([res.results[c]["y"] for c in core_ids])  # [NC, P, NT, 2, Wt]
    yr = yy.astype(np.float32).transpose(0, 2, 1, 3, 4)  # [NC, NT, P, 2, Wt]
    yr = yr.reshape(NCORES, BPC, SEG, NDF, 2, Wt)
    re = yr[..., 0, :]  # [NC, BPC, SEG, 96, Wt]
    im = yr[..., 1, :]
    re = re.reshape(B, SEG, NDF, Wt).transpose(0, 1, 3, 2).reshape(B, T, NDF)
    im = im.reshape(B, SEG, NDF, Wt).transpose(0, 1, 3, 2).reshape(B, T, NDF)
    out = np.array(spec, dtype=np.float32, copy=True)
    out[:, 0, :, :NDF, 0] = re
    out[:, 0, :, :NDF, 1] = im
    return out


# revision 4
# speedup vs baseline: 1.1129x; 1.1129x over previous
"""Trainium2 Bass kernel for nn_DfOpStrided — v8 (multi-engine split).

Math (reference):
    x[t] = spec[:, 0, t, :96, :]                     (complex, [T, 96])
    spec_f[t] = sum_k c[t, k] * x[t + k - 4]         (complex MAC, zero-pad t<0)
    out[t] = alpha[t] * spec_f[t] + (1 - alpha[t]) * x[t]

Host-side fusion: out[t] = sum_k chat[t, k] * x[t+k-4] with
    chat[t, k] = alpha[t] * c[t, k]  (+ (1-alpha[t]) on Re(chat[t, 4])).

Engine split (v3 was DVE-only and DVE-bound):
  - DVE: tap 0-3 products (plain complex mult), plus the final re/im
    combine. All ops hit the fp16 2x DVE mode (4B-aligned, step 1).
  - GpSimd: tap-4 products (alignment-agnostic, ~2.2 ns/col).
  - ACT: builds the odd-parity shifted x copy (for taps 1/3 alignment),
    copies PSUM sums back to SBUF fp16, runs the x/y DMA ring.
  - PE: sums the 5 tap-product planes via identity-stationary matmuls
    accumulating in PSUM (replaces the DVE accumulate-add chain).
    Identity never changes -> ldweights=False on all but the first
    matmul (skips 120 redundant weight loads).
  - Sync: coef DMA ring (HWDGE).

Products per tap (plain complex):
    P0 = cr*xr, P1 = ci*xi, P2 = cr*xi, P3 = ci*xr
    re = P0 - P1, im = P2 + P3

Packing: row = one (batch, time-segment, freq-bin) triple, Wt=1000
samples per segment + 4-sample halo. 2*2*96 = 384 rows per core = three
128-row tiles. PSUM bank = 512 fp32 -> matmuls work on 500-col halves.
Pure data-parallel over batch: 16 batches -> 8 cores x 2.
"""

import sys

sys.path.insert(0, "/opt/trn_rl_repo")

import numpy as np
from concourse import bass, bacc, tile, mybir
from concourse.bass_utils import run_bass_kernel_spmd

B, T, F, NDF, ORDER = 16, 2000, 481, 96, 5
NCORES = 8
BPC = B // NCORES  # batches per core
PAD = ORDER - 1  # causal zero-pad
Wt = 1000  # samples per row segment
SEG = T // Wt  # segments per batch
XW = Wt + PAD  # x row window (halo)
HW_ = 500  # matmul half-segment (one PSUM bank)
P = 128
ROWS = BPC * SEG * NDF  # 384 rows per core
NT = ROWS // P  # 3 tiles per core

_cache: dict = {}


def _build():
    if "nc" in _cache:
        return _cache["nc"]
    f32 = mybir.dt.float32
    dt = mybir.dt.float16
    mult = mybir.AluOpType.mult
    nc = bacc.Bacc("TRN2", target_bir_lowering=False, debug=False, num_devices=NCORES)
    xin = nc.dram_tensor("xin", [P, NT, 2, XW], dt, kind="ExternalInput")
    coef = nc.dram_tensor("coef", [P, NT, ORDER, 2, Wt], dt, kind="ExternalInput")
    ident = nc.dram_tensor("ident", [P, P], dt, kind="ExternalInput")
    y = nc.dram_tensor("y", [P, NT, 2, Wt], dt, kind="ExternalOutput")

    first_mm = [True]

    def mm(out, lhsT, rhs, start, stop):
        inst = nc.tensor.matmul(out, lhsT, rhs, start=start, stop=stop)
        if first_mm[0]:
            first_mm[0] = False
        else:
            inst.ldweights = False
        return inst

    with tile.TileContext(nc) as tc:
        with (
            tc.tile_pool(name="xp", bufs=1) as xpool,
            tc.tile_pool(name="cp", bufs=2) as cpool,
            tc.tile_pool(name="pp", bufs=2) as ppool,
            tc.psum_pool(name="ps", bufs=2) as pspool,
            tc.tile_pool(name="ab", bufs=3) as apool,
            tc.tile_pool(name="yp", bufs=2) as ypool,
        ):
            idt = xpool.tile([P, P], dt, tag="ident")
            nc.scalar.dma_start(out=idt[:, :], in_=ident[:, :])
            xt = xpool.tile([P, NT, 2, XW], dt, tag="x")
            nc.scalar.dma_start(out=xt[:, :], in_=xin[:, :])
            # odd-parity shifted copy, built by the ACT engine (it has slack;
            # gp-built copies measured 3.5 ns/col in v7 and serialized the fill)
            xo = xpool.tile([P, NT, 2, XW], dt, tag="xodd")
            for i in range(NT):
                nc.scalar.activation(
                    xo[:, i, :, 0 : XW - 1],
                    xt[:, i, :, 1:XW],
                    mybir.ActivationFunctionType.Copy,
                )
            for i in range(NT):
                ck = cpool.tile([P, ORDER, 2, Wt], dt, tag="ck")
                nc.sync.dma_start(out=ck[:, :], in_=coef[:, i])
                prods = []
                for k in range(ORDER):
                    pk = ppool.tile([P, 4, Wt], dt, tag=f"prod{k}")
                    eng = nc.vector if k < 4 else nc.gpsimd
                    par = k % 2
                    src = xt if par == 0 else xo
                    off = k - par  # even offset into src
                    # (cr, ci) x (xr, xi) -> P0, P1 ; (cr, ci) x (xi, xr) -> P2, P3
                    eng.tensor_tensor(
                        pk[:, 0, :], ck[:, k, 0, :], src[:, i, 0, off : off + Wt], mult
                    )
                    eng.tensor_tensor(
                        pk[:, 1, :], ck[:, k, 1, :], src[:, i, 1, off : off + Wt], mult
                    )
                    eng.tensor_tensor(
                        pk[:, 2, :], ck[:, k, 0, :], src[:, i, 1, off : off + Wt], mult
                    )
                    eng.tensor_tensor(
                        pk[:, 3, :], ck[:, k, 1, :], src[:, i, 0, off : off + Wt], mult
                    )
                    prods.append(pk)
                yt = ypool.tile([P, 2, Wt], dt, tag="y")
                for h in range(2):
                    ps = pspool.tile([P, 4, 512], f32, tag="psum")
                    for k in range(ORDER):
                        for s in range(4):
                            mm(
                                ps[:, s, 0:HW_],
                                idt[:, :],
                                prods[k][:, s, h * HW_ : h * HW_ + HW_],
                                start=(k == 0),
                                stop=(k == ORDER - 1),
                            )
                    abc = apool.tile([P, 4, HW_], dt, tag="abc")
                    nc.scalar.activation(
                        abc[:, :], ps[:, :, 0:HW_], mybir.ActivationFunctionType.Copy
                    )
                    nc.vector.tensor_tensor(
                        yt[:, 0, h * HW_ : h * HW_ + HW_],
                        abc[:, 0, :],
                        abc[:, 1, :],
                        mybir.AluOpType.subtract,
                    )
                    nc.vector.tensor_tensor(
                        yt[:, 1, h * HW_ : h * HW_ + HW_],
                        abc[:, 2, :],
                        abc[:, 3, :],
                        mybir.AluOpType.add,
                    )
                nc.scalar.dma_start(out=y[:, i], in_=yt[:, :])
    nc.compile()
    _cache["nc"] = nc
    return nc


def _host_prep(spec, coefs, alpha):
    """Build per-core xin/coef arrays (all cores at once)."""
    spec32 = np.asarray(spec, dtype=np.float32)
    coefs32 = np.asarray(coefs, dtype=np.float32)
    alpha32 = np.asarray(alpha, dtype=np.float32)

    x = spec32[:, 0, :, :NDF, :]  # [B, T, 96, 2]
    xpad = np.zeros((B, 2, NDF, PAD + T), dtype=np.float32)
    xpad[:, 0, :, PAD:] = x[..., 0].transpose(0, 2, 1)
    xpad[:, 1, :, PAD:] = x[..., 1].transpose(0, 2, 1)
    xpad = xpad.astype(np.float16)
    # per-segment windows with halo -> rows (b, seg, f)
    xw = np.stack([xpad[:, :, :, s * Wt : s * Wt + XW] for s in range(SEG)], axis=1)
    xw = xw.transpose(0, 1, 3, 2, 4)  # [B, SEG, 96, 2, XW]
    xin_all = (
        xw.reshape(NCORES, NT, P, 2, XW).transpose(0, 2, 1, 3, 4).copy()
    )  # [NC, P, NT, 2, XW]

    a = alpha32[:, :, 0]  # [B, T]
    ca = coefs32 * a[:, :, None, None, None]  # [B, T, 5, 96, 2]
    ca[:, :, ORDER - 1, :, 0] += (1.0 - a)[:, :, None]
    ca = ca.astype(np.float16)
    cs = ca.reshape(B, SEG, Wt, ORDER, NDF, 2).transpose(
        0, 1, 4, 3, 5, 2
    )  # [B, SEG, 96, 5, 2, Wt]
    coef_all = (
        cs.reshape(NCORES, NT, P, ORDER, 2, Wt).transpose(0, 2, 1, 3, 4, 5).copy()
    )  # [NC, P, NT, 5, 2, Wt]

    ident = np.eye(P, dtype=np.float16)
    return xin_all, coef_all, ident


def kernel(spec, coefs, alpha, _bass_results_hook=None):
    nc = _build()
    xin_all, coef_all, ident = _host_prep(spec, coefs, alpha)

    core_ids = list(range(NCORES))
    in_maps = [
        {"xin": xin_all[c], "coef": coef_all[c], "ident": ident} for c in core_ids
    ]
    res = run_bass_kernel_spmd(nc, in_maps, core_ids)
    if _bass_results_hook is not None:
        _bass_results_hook(res)

    yy = np.stack([res.results[c]["y"] for c in core_ids])  # [NC, P, NT, 2, Wt]
    yr = yy.astype(np.float32).transpose(0, 2, 1, 3, 4)  # [NC, NT, P, 2, Wt]
    yr = yr.reshape(NCORES, BPC, SEG, NDF, 2, Wt)
    re = yr[..., 0, :]  # [NC, BPC, SEG, 96, Wt]
    im = yr[..., 1, :]
    re = re.reshape(B, SEG, NDF, Wt).transpose(0, 1, 3, 2).reshape(B, T, NDF)
    im = im.reshape(B, SEG, NDF, Wt).transpose(0, 1, 3, 2).reshape(B, T, NDF)
    out = np.array(spec, dtype=np.float32, copy=True)
    out[:, 0, :, :NDF, 0] = re
    out[:, 0, :, :NDF, 1] = im
    return out


# revision 6
# speedup vs baseline: 1.3918x; 1.2506x over previous
"""Trainium2 Bass kernel for nn_DfOpStrided — v8 (multi-engine split).

Math (reference):
    x[t] = spec[:, 0, t, :96, :]                     (complex, [T, 96])
    spec_f[t] = sum_k c[t, k] * x[t + k - 4]         (complex MAC, zero-pad t<0)
    out[t] = alpha[t] * spec_f[t] + (1 - alpha[t]) * x[t]

Host-side fusion: out[t] = sum_k chat[t, k] * x[t+k-4] with
    chat[t, k] = alpha[t] * c[t, k]  (+ (1-alpha[t]) on Re(chat[t, 4])).

Engine split (v3 was DVE-only and DVE-bound):
  - DVE: tap 0-3 products (plain complex mult), plus the final re/im
    combine. All ops hit the fp16 2x DVE mode (4B-aligned, step 1).
  - GpSimd: tap-4 products (alignment-agnostic, ~2.2 ns/col).
  - ACT: builds the odd-parity shifted x copy (for taps 1/3 alignment),
    copies PSUM sums back to SBUF fp16, runs the x/y DMA ring.
  - PE: sums the 5 tap-product planes via identity-stationary matmuls
    accumulating in PSUM (replaces the DVE accumulate-add chain).
    Identity never changes -> ldweights=False on all but the first
    matmul (skips 120 redundant weight loads).
  - Sync: coef DMA ring (HWDGE).

Products per tap (plain complex):
    P0 = cr*xr, P1 = ci*xi, P2 = cr*xi, P3 = ci*xr
    re = P0 - P1, im = P2 + P3

Packing: row = one (batch, time-segment, freq-bin) triple, Wt=1000
samples per segment + 4-sample halo. 2*2*96 = 384 rows per core = three
128-row tiles. PSUM bank = 512 fp32 -> matmuls work on 500-col halves.
Pure data-parallel over batch: 16 batches -> 8 cores x 2.
"""

import sys

sys.path.insert(0, "/opt/trn_rl_repo")

import numpy as np
from concourse import bass, bacc, tile, mybir
from concourse.bass_utils import run_bass_kernel_spmd

B, T, F, NDF, ORDER = 16, 2000, 481, 96, 5
NCORES = 8
BPC = B // NCORES  # batches per core
PAD = ORDER - 1  # causal zero-pad
Wt = 1000  # samples per row segment
SEG = T // Wt  # segments per batch
XW = Wt + PAD  # x row window (halo)
HW_ = 500  # matmul half-segment (one PSUM bank)
P = 128
ROWS = BPC * SEG * NDF  # 384 rows per core
NT = ROWS // P  # 3 tiles per core

_cache: dict = {}


def _build():
    if "nc" in _cache:
        return _cache["nc"]
    f32 = mybir.dt.float32
    dt = mybir.dt.float16
    mult = mybir.AluOpType.mult
    nc = bacc.Bacc("TRN2", target_bir_lowering=False, debug=False, num_devices=NCORES)
    # x planes tripled (xr, xi, xr): both (xr,xi) and (xi,xr) plane pairs are
    # contiguous slices, so each tap is 2 paired DVE ops instead of 4
    xin = nc.dram_tensor("xin", [P, NT, 3, XW], dt, kind="ExternalInput")
    coef = nc.dram_tensor("coef", [P, NT, ORDER, 2, Wt], dt, kind="ExternalInput")
    ident = nc.dram_tensor("ident", [P, P], dt, kind="ExternalInput")
    y = nc.dram_tensor("y", [P, NT, 2, Wt], dt, kind="ExternalOutput")

    with tile.TileContext(nc) as tc:
        with (
            tc.tile_pool(name="xp", bufs=1) as xpool,
            tc.tile_pool(name="cp", bufs=2) as cpool,
            tc.tile_pool(name="pp", bufs=2) as ppool,
            tc.psum_pool(name="ps", bufs=2) as pspool,
            tc.tile_pool(name="ab", bufs=3) as apool,
            tc.tile_pool(name="yp", bufs=2) as ypool,
        ):
            idt = xpool.tile([P, P], dt, tag="ident")
            nc.scalar.dma_start(out=idt[:, :], in_=ident[:, :])
            xt = xpool.tile([P, NT, 3, XW], dt, tag="x")
            for i in range(NT):
                nc.scalar.dma_start(out=xt[:, i], in_=xin[:, i])
            # odd-parity shifted copy, built by the ACT engine (it has slack;
            # gp-built copies measured 3.5 ns/col in v7 and serialized the fill)
            xo = xpool.tile([P, NT, 3, XW], dt, tag="xodd")
            for i in range(NT):
                nc.scalar.activation(
                    xo[:, i, :, 0 : XW - 1],
                    xt[:, i, :, 1:XW],
                    mybir.ActivationFunctionType.Copy,
                )
            for i in range(NT):
                ck = cpool.tile([P, ORDER, 2, Wt], dt, tag="ck")
                nc.sync.dma_start(out=ck[:, :], in_=coef[:, i])
                prods = []
                for k in range(ORDER):
                    pk = ppool.tile([P, 4, Wt], dt, tag=f"prod{k}")
                    par = k % 2
                    src = xt if par == 0 else xo
                    off = k - par  # even offset into src
                    # (cr, ci) x (xr, xi) -> P0, P1 ; (cr, ci) x (xi, xr) -> P2, P3
                    nc.vector.tensor_tensor(
                        pk[:, 0:2, :],
                        ck[:, k, :, :],
                        src[:, i, 0:2, off : off + Wt],
                        mult,
                    )
                    nc.vector.tensor_tensor(
                        pk[:, 2:4, :],
                        ck[:, k, :, :],
                        src[:, i, 1:3, off : off + Wt],
                        mult,
                    )
                    prods.append(pk)
                yt = ypool.tile([P, 2, Wt], dt, tag="y")
                for h in range(2):
                    ps = pspool.tile([P, 4, 512], f32, tag="psum")
                    for k in range(ORDER):
                        for s in range(4):
                            nc.tensor.matmul(
                                ps[:, s, 0:HW_],
                                idt[:, :],
                                prods[k][:, s, h * HW_ : h * HW_ + HW_],
                                start=(k == 0),
                                stop=(k == ORDER - 1),
                            )
                    abc = apool.tile([P, 4, HW_], dt, tag="abc")
                    nc.scalar.activation(
                        abc[:, :], ps[:, :, 0:HW_], mybir.ActivationFunctionType.Copy
                    )
                    nc.vector.tensor_tensor(
                        yt[:, 0, h * HW_ : h * HW_ + HW_],
                        abc[:, 0, :],
                        abc[:, 1, :],
                        mybir.AluOpType.subtract,
                    )
                    nc.vector.tensor_tensor(
                        yt[:, 1, h * HW_ : h * HW_ + HW_],
                        abc[:, 2, :],
                        abc[:, 3, :],
                        mybir.AluOpType.add,
                    )
                nc.sync.dma_start(out=y[:, i], in_=yt[:, :])
    nc.compile()
    _cache["nc"] = nc
    return nc


def _host_prep(spec, coefs, alpha):
    """Build per-core xin/coef arrays (all cores at once)."""
    spec32 = np.asarray(spec, dtype=np.float32)
    coefs32 = np.asarray(coefs, dtype=np.float32)
    alpha32 = np.asarray(alpha, dtype=np.float32)

    x = spec32[:, 0, :, :NDF, :]  # [B, T, 96, 2]
    xpad = np.zeros((B, 3, NDF, PAD + T), dtype=np.float32)
    xpad[:, 0, :, PAD:] = x[..., 0].transpose(0, 2, 1)
    xpad[:, 1, :, PAD:] = x[..., 1].transpose(0, 2, 1)
    xpad[:, 2] = xpad[:, 0]  # tripled planes (xr, xi, xr)
    xpad = xpad.astype(np.float16)
    # per-segment windows with halo -> rows (b, seg, f)
    xw = np.stack([xpad[:, :, :, s * Wt : s * Wt + XW] for s in range(SEG)], axis=1)
    xw = xw.transpose(0, 1, 3, 2, 4)  # [B, SEG, 96, 3, XW]
    xin_all = (
        xw.reshape(NCORES, NT, P, 3, XW).transpose(0, 2, 1, 3, 4).copy()
    )  # [NC, P, NT, 3, XW]

    a = alpha32[:, :, 0]  # [B, T]
    ca = coefs32 * a[:, :, None, None, None]  # [B, T, 5, 96, 2]
    ca[:, :, ORDER - 1, :, 0] += (1.0 - a)[:, :, None]
    ca = ca.astype(np.float16)
    cs = ca.reshape(B, SEG, Wt, ORDER, NDF, 2).transpose(
        0, 1, 4, 3, 5, 2
    )  # [B, SEG, 96, 5, 2, Wt]
    coef_all = (
        cs.reshape(NCORES, NT, P, ORDER, 2, Wt).transpose(0, 2, 1, 3, 4, 5).copy()
    )  # [NC, P, NT, 5, 2, Wt]

    ident = np.eye(P, dtype=np.float16)
    return xin_all, coef_all, ident


def kernel(spec, coefs, alpha, _bass_results_hook=None):
    nc = _build()
    xin_all, coef_all, ident = _host_prep(spec, coefs, alpha)

    core_ids = list(range(NCORES))
    in_maps = [
        {"xin": xin_all[c], "coef": coef_all[c], "ident": ident} for c in core_ids
    ]
    res = run_bass_kernel_spmd(nc, in_maps, core_ids)
    if _bass_results_hook is not None:
        _bass_results_hook(res)

    yy = np.stack([res.results[c]["y"] for c in core_ids])  # [NC, P, NT, 2, Wt]
    yr = yy.astype(np.float32).transpose(0, 2, 1, 3, 4)  # [NC, NT, P, 2, Wt]
    yr = yr.reshape(NCORES, BPC, SEG, NDF, 2, Wt)
    re = yr[..., 0, :]  # [NC, BPC, SEG, 96, Wt]
    im = yr[..., 1, :]
    re = re.reshape(B, SEG, NDF, Wt).transpose(0, 1, 3, 2).reshape(B, T, NDF)
    im = im.reshape(B, SEG, NDF, Wt).transpose(0, 1, 3, 2).reshape(B, T, NDF)
    out = np.array(spec, dtype=np.float32, copy=True)
    out[:, 0, :, :NDF, 0] = re
    out[:, 0, :, :NDF, 1] = im
    return out


# revision 9
# speedup vs baseline: 1.4970x; 1.0755x over previous
"""Trainium2 Bass kernel for nn_DfOpStrided — v8 (multi-engine split).

Math (reference):
    x[t] = spec[:, 0, t, :96, :]                     (complex, [T, 96])
    spec_f[t] = sum_k c[t, k] * x[t + k - 4]         (complex MAC, zero-pad t<0)
    out[t] = alpha[t] * spec_f[t] + (1 - alpha[t]) * x[t]

Host-side fusion: out[t] = sum_k chat[t, k] * x[t+k-4] with
    chat[t, k] = alpha[t] * c[t, k]  (+ (1-alpha[t]) on Re(chat[t, 4])).

Engine split (v3 was DVE-only and DVE-bound):
  - DVE: tap 0-3 products (plain complex mult), plus the final re/im
    combine. All ops hit the fp16 2x DVE mode (4B-aligned, step 1).
  - GpSimd: tap-4 products (alignment-agnostic, ~2.2 ns/col).
  - ACT: builds the odd-parity shifted x copy (for taps 1/3 alignment),
    copies PSUM sums back to SBUF fp16, runs the x/y DMA ring.
  - PE: sums the 5 tap-product planes via identity-stationary matmuls
    accumulating in PSUM (replaces the DVE accumulate-add chain).
    Identity never changes -> ldweights=False on all but the first
    matmul (skips 120 redundant weight loads).
  - Sync: coef DMA ring (HWDGE).

Products per tap (plain complex):
    P0 = cr*xr, P1 = ci*xi, P2 = cr*xi, P3 = ci*xr
    re = P0 - P1, im = P2 + P3

Packing: row = one (batch, time-segment, freq-bin) triple, Wt=1000
samples per segment + 4-sample halo. 2*2*96 = 384 rows per core = three
128-row tiles. PSUM bank = 512 fp32 -> matmuls work on 500-col halves.
Pure data-parallel over batch: 16 batches -> 8 cores x 2.
"""

import sys

sys.path.insert(0, "/opt/trn_rl_repo")

import numpy as np
from concourse import bass, bacc, tile, mybir
from concourse.bass_utils import run_bass_kernel_spmd

B, T, F, NDF, ORDER = 16, 2000, 481, 96, 5
NCORES = 8
BPC = B // NCORES  # batches per core
PAD = ORDER - 1  # causal zero-pad
Wt = 1000  # samples per row segment
SEG = T // Wt  # segments per batch
XW = Wt + PAD  # x row window (halo)
HW_ = 500  # matmul half-segment (one PSUM bank)
P = 128
ROWS = BPC * SEG * NDF  # 384 rows per core
NT = ROWS // P  # 3 tiles per core

_cache: dict = {}


def _prune_ldweights(nc):
    """Drop redundant PE weight loads.

    The tile legalizer emits one InstLdweights per matmul even when the
    stationary operand never changes (our identity). Each reload costs
    ~100 ns of PE queue time. Keep an LDW if it carries a wait, if its
    weights differ from the previous kept LDW, or if its matmult has a
    wait (compile()'s move_matmul_waits_to_ldweights may need the LDW
    as a wait slot); drop the rest.
    """
    for fn in nc.m.functions:
        for blk in fn.blocks:
            insts = list(blk.instructions)
            keep = []
            last_sig = None
            n = len(insts)
            changed = False
            for idx, ins in enumerate(insts):
                if isinstance(ins, mybir.InstLdweights):
                    ap = ins.ins[0]
                    sig = (ap.memref, ap.offset, str(ap.ap))
                    nxt = insts[idx + 1] if idx + 1 < n else None
                    if (
                        sig == last_sig
                        and not ins.has_wait()
                        and isinstance(nxt, mybir.InstMatmult)
                        and not nxt.has_wait()
                    ):
                        changed = True
                        continue
                    last_sig = sig
                keep.append(ins)
            if changed:
                blk.instructions = keep


def _build():
    if "nc" in _cache:
        return _cache["nc"]
    f32 = mybir.dt.float32
    dt = mybir.dt.float16
    mult = mybir.AluOpType.mult
    nc = bacc.Bacc("TRN2", target_bir_lowering=False, debug=False, num_devices=NCORES)
    # x planes tripled (xr, xi, xr): both (xr,xi) and (xi,xr) plane pairs are
    # contiguous slices, so each tap is 2 paired DVE ops instead of 4
    xin = nc.dram_tensor("xin", [P, NT, 3, XW], dt, kind="ExternalInput")
    coef = nc.dram_tensor("coef", [P, NT, ORDER, 2, Wt], dt, kind="ExternalInput")
    ident = nc.dram_tensor("ident", [P, P], dt, kind="ExternalInput")
    y = nc.dram_tensor("y", [P, NT, 2, Wt], dt, kind="ExternalOutput")

    with tile.TileContext(nc) as tc:
        with (
            tc.tile_pool(name="xp", bufs=1) as xpool,
            tc.tile_pool(name="cp", bufs=2) as cpool,
            tc.tile_pool(name="pp", bufs=2) as ppool,
            tc.psum_pool(name="ps", bufs=2) as pspool,
            tc.tile_pool(name="ab", bufs=3) as apool,
            tc.tile_pool(name="yp", bufs=2) as ypool,
        ):
            idt = xpool.tile([P, P], dt, tag="ident")
            nc.scalar.dma_start(out=idt[:, :], in_=ident[:, :])
            xt = xpool.tile([P, NT, 3, XW], dt, tag="x")
            for i in range(NT):
                nc.scalar.dma_start(out=xt[:, i], in_=xin[:, i])
            # odd-parity shifted copy, built by the ACT engine (it has slack;
            # gp-built copies measured 3.5 ns/col in v7 and serialized the fill)
            xo = xpool.tile([P, NT, 3, XW], dt, tag="xodd")
            for i in range(NT):
                nc.scalar.activation(
                    xo[:, i, :, 0 : XW - 1],
                    xt[:, i, :, 1:XW],
                    mybir.ActivationFunctionType.Copy,
                )
            for i in range(NT):
                ck = cpool.tile([P, ORDER, 2, Wt], dt, tag="ck")
                # two chunks so tile-0 products can start ~4 us earlier
                nc.sync.dma_start(out=ck[:, 0:3], in_=coef[:, i, 0:3])
                nc.sync.dma_start(out=ck[:, 3:5], in_=coef[:, i, 3:5])
                prods = []
                for k in range(ORDER):
                    pk = ppool.tile([P, 4, Wt], dt, tag=f"prod{k}")
                    par = k % 2
                    src = xt if par == 0 else xo
                    off = k - par  # even offset into src
                    # (cr, ci) x (xr, xi) -> P0, P1 ; (cr, ci) x (xi, xr) -> P2, P3
                    nc.vector.tensor_tensor(
                        pk[:, 0:2, :],
                        ck[:, k, :, :],
                        src[:, i, 0:2, off : off + Wt],
                        mult,
                    )
                    nc.vector.tensor_tensor(
                        pk[:, 2:4, :],
                        ck[:, k, :, :],
                        src[:, i, 1:3, off : off + Wt],
                        mult,
                    )
                    prods.append(pk)
                yt = ypool.tile([P, 2, Wt], dt, tag="y")
                for h in range(2):
                    ps = pspool.tile([P, 4, 512], f32, tag="psum")
                    for k in range(ORDER):
                        for s in range(4):
                            nc.tensor.matmul(
                                ps[:, s, 0:HW_],
                                idt[:, :],
                                prods[k][:, s, h * HW_ : h * HW_ + HW_],
                                start=(k == 0),
                                stop=(k == ORDER - 1),
                            )
                    abc = apool.tile([P, 4, HW_], dt, tag="abc")
                    nc.scalar.activation(
                        abc[:, :], ps[:, :, 0:HW_], mybir.ActivationFunctionType.Copy
                    )
                    nc.vector.tensor_tensor(
                        yt[:, 0, h * HW_ : h * HW_ + HW_],
                        abc[:, 0, :],
                        abc[:, 1, :],
                        mybir.AluOpType.subtract,
                    )
                    nc.vector.tensor_tensor(
                        yt[:, 1, h * HW_ : h * HW_ + HW_],
                        abc[:, 2, :],
                        abc[:, 3, :],
                        mybir.AluOpType.add,
                    )
                nc.sync.dma_start(out=y[:, i], in_=yt[:, :])
    _prune_ldweights(nc)
    nc.compile()
    _cache["nc"] = nc
    return nc


def _host_prep(spec, coefs, alpha):
    """Build per-core xin/coef arrays (all cores at once)."""
    spec32 = np.asarray(spec, dtype=np.float32)
    coefs32 = np.asarray(coefs, dtype=np.float32)
    alpha32 = np.asarray(alpha, dtype=np.float32)

    x = spec32[:, 0, :, :NDF, :]  # [B, T, 96, 2]
    xpad = np.zeros((B, 3, NDF, PAD + T), dtype=np.float32)
    xpad[:, 0, :, PAD:] = x[..., 0].transpose(0, 2, 1)
    xpad[:, 1, :, PAD:] = x[..., 1].transpose(0, 2, 1)
    xpad[:, 2] = xpad[:, 0]  # tripled planes (xr, xi, xr)
    xpad = xpad.astype(np.float16)
    # per-segment windows with halo -> rows (b, seg, f)
    xw = np.stack([xpad[:, :, :, s * Wt : s * Wt + XW] for s in range(SEG)], axis=1)
    xw = xw.transpose(0, 1, 3, 2, 4)  # [B, SEG, 96, 3, XW]
    xin_all = (
        xw.reshape(NCORES, NT, P, 3, XW).transpose(0, 2, 1, 3, 4).copy()
    )  # [NC, P, NT, 3, XW]

    a = alpha32[:, :, 0]  # [B, T]
    ca = coefs32 * a[:, :, None, None, None]  # [B, T, 5, 96, 2]
    ca[:, :, ORDER - 1, :, 0] += (1.0 - a)[:, :, None]
    ca = ca.astype(np.float16)
    cs = ca.reshape(B, SEG, Wt, ORDER, NDF, 2).transpose(
        0, 1, 4, 3, 5, 2
    )  # [B, SEG, 96, 5, 2, Wt]
    coef_all = (
        cs.reshape(NCORES, NT, P, ORDER, 2, Wt).transpose(0, 2, 1, 3, 4, 5).copy()
    )  # [NC, P, NT, 5, 2, Wt]

    ident = np.eye(P, dtype=np.float16)
    return xin_all, coef_all, ident


def kernel(spec, coefs, alpha, _bass_results_hook=None):
    nc = _build()
    xin_all, coef_all, ident = _host_prep(spec, coefs, alpha)

    core_ids = list(range(NCORES))
    in_maps = [
        {"xin": xin_all[c], "coef": coef_all[c], "ident": ident} for c in core_ids
    ]
    res = run_bass_kernel_spmd(nc, in_maps, core_ids)
    if _bass_results_hook is not None:
        _bass_results_hook(res)

    yy = np.stack([res.results[c]["y"] for c in core_ids])  # [NC, P, NT, 2, Wt]
    yr = yy.astype(np.float32).transpose(0, 2, 1, 3, 4)  # [NC, NT, P, 2, Wt]
    yr = yr.reshape(NCORES, BPC, SEG, NDF, 2, Wt)
    re = yr[..., 0, :]  # [NC, BPC, SEG, 96, Wt]
    im = yr[..., 1, :]
    re = re.reshape(B, SEG, NDF, Wt).transpose(0, 1, 3, 2).reshape(B, T, NDF)
    im = im.reshape(B, SEG, NDF, Wt).transpose(0, 1, 3, 2).reshape(B, T, NDF)
    out = np.array(spec, dtype=np.float32, copy=True)
    out[:, 0, :, :NDF, 0] = re
    out[:, 0, :, :NDF, 1] = im
    return out
